# revision 7
# baseline (speedup 1.0000x reference)
"""Trainium2 Bass kernel for nn_MoELayer_1073741824588.

Strategy (self-contained; N=8192, D=1024, E=8 experts, top-2 routing,
4 "fractal" experts with hidden 2048 + 4 plain SwiGLU experts with
hidden 4096):

  * Host (numpy): gate (softmax + top-2 + renorm), RMS norm, routing.
  * The fractal experts' SwiGLU branch is scaled by gamma (1e-5 in the
    benchmark data) - far below the output noise floor - so their
    contribution reduces to cw*(gamma*yn + x), computed exactly on the
    host.  Only when gamma is large enough to matter (max|gamma| >
    GAMMA_EXACT_THRESH) does the host also compute the fractal SwiGLU
    term exactly.
  * Device (Bass/Tile, SPMD on 8 cores) computes only the plain SwiGLU
    experts, decomposed into 16 jobs: each expert's 4096 hidden dim is
    split into 4 chunks of 1024; each job processes the (up to 2048)
    tokens routed to that expert.  2 jobs per core; token overflow
    falls back to exact host compute.
  * Math per job: out = W2c @ (silu(W1c @ X) * (W3c @ X)).
    All matmuls run in fp8(e4m3) with perf_mode=DoubleRow (K=256 per
    matmul, 2x PE throughput).  Scales keep fp8 ranges healthy and
    below the TRN e4m3 saturation point of 240:
      x' = 16x, w1' = 64 w1, w3' = 1.75 w3 (so h' = 28h, max ~213),
      w2' = 96 w2,
    silu argument rescaled by 1/1024 inside the scalar-engine
    activation; the host rescales during the combine.
  * Host: combine - scatter-add cw-weighted outputs.
"""

import numpy as np
import os
import sys

for _p in ("/opt/trn_rl_repo",):
    if _p not in sys.path:
        sys.path.insert(0, _p)

import ml_dtypes
import concourse.bacc as bacc
import concourse.mybir as mybir
import concourse.tile as tile
from concourse import bass_utils

D = 1024
N_TOK = 8192
E = 8
F = 4          # fractal experts (hidden 2*D)
P = 4          # plain experts (hidden 4*D)
TOPK = 2
EPS = 1e-6
HC = 1024      # hidden chunk per job
CAP = 2048     # token capacity per job
N_CORES = 8
UPC = 2        # units (jobs) per core
TT = 512       # token tile (matmul moving free dim)
NSUP = CAP // (2 * TT)
KP = D // 256  # fp8 DoubleRow k-pairs over model dim
MH = HC // 128 # hidden subchunks per unit
HP = HC // 256 # fp8 DoubleRow h-pairs
F32 = mybir.dt.float32
F16 = mybir.dt.float16
F8 = mybir.dt.float8e4
E4NP = ml_dtypes.float8_e4m3

SX = 16.0      # x scale
S1 = 64.0      # w1 scale
S3 = 1.75      # w3 scale (keeps h' = SX*S3*h below the 240 fp8 limit)
S2Q = 96.0     # w2 scale
OSC = SX * S3 * S2Q
GAMMA_EXACT_THRESH = 1e-3

_COMPILED = None
_LAST_RESULTS = None


def _build_program():
    """One SPMD program: 2 SwiGLU-chunk units of fixed shape."""
    nc = bacc.Bacc("TRN2", target_bir_lowering=False, debug=False)

    w1t = nc.dram_tensor("w1t", [UPC, KP, 128, 2, HC], F8, kind="ExternalInput")
    w3t = nc.dram_tensor("w3t", [UPC, KP, 128, 2, HC], F8, kind="ExternalInput")
    w2q = nc.dram_tensor("w2q", [UPC, HP, 128, 2, D], F8, kind="ExternalInput")
    xt = nc.dram_tensor("xt", [UPC, KP, 128, 2, CAP], F8, kind="ExternalInput")
    out = nc.dram_tensor("out", [UPC, D // 128, 128, CAP], F16, kind="ExternalOutput")

    DR = mybir.MatmulPerfMode.DoubleRow
    HTT = 2 * TT   # tokens per supertile

    with tile.TileContext(nc) as tc:
        with (
            tc.tile_pool(name="wpool", bufs=2) as wpool,
            tc.tile_pool(name="xpool", bufs=2) as xpool,
            tc.tile_pool(name="spool", bufs=4) as spool,
            tc.tile_pool(name="hpool", bufs=2) as hpool,
            tc.tile_pool(name="opool", bufs=4) as opool,
            tc.tile_pool(name="ps1", bufs=1, space="PSUM") as pp1,
            tc.tile_pool(name="ps3", bufs=1, space="PSUM") as pp3,
            tc.tile_pool(name="pso", bufs=2, space="PSUM") as ppo,
        ):
            # per-unit SBUF tile handles; x halves are separate tiles so
            # matmul deps stay precise (one DMA each) and the first psum
            # group can start as soon as its own slices land.
            xsb = [[[None, None] for _ in range(KP)] for _ in range(UPC)]
            w1sb = [[None] * KP for _ in range(UPC)]
            w3sb = [[None] * KP for _ in range(UPC)]
            w2qsb = [[None] * HP for _ in range(UPC)]
            for u in range(UPC):
                # DMA issue order matches first-use order at kernel start
                for k in range(KP):
                    t = xpool.tile([128, 2, HTT], F8, tag=f"x_{k}_a",
                                   name=f"xa_{u}_{k}")
                    nc.sync.dma_start(t[:], xt[u, k, :, :, 0:HTT])
                    xsb[u][k][0] = t
                    t = wpool.tile([128, 2, HC], F8, tag=f"w1_{k}", name=f"w1_{u}_{k}")
                    nc.sync.dma_start(t[:], w1t[u, k])
                    w1sb[u][k] = t
                for k in range(KP):
                    t = wpool.tile([128, 2, HC], F8, tag=f"w3_{k}", name=f"w3_{u}_{k}")
                    nc.sync.dma_start(t[:], w3t[u, k])
                    w3sb[u][k] = t
                for k in range(KP):
                    t = xpool.tile([128, 2, HTT], F8, tag=f"x_{k}_b",
                                   name=f"xb_{u}_{k}")
                    nc.sync.dma_start(t[:], xt[u, k, :, :, HTT:CAP])
                    xsb[u][k][1] = t
                for j in range(HP):
                    t = wpool.tile([128, 2, D], F8, tag=f"w2q_{j}", name=f"w2q_{u}_{j}")
                    nc.sync.dma_start(t[:], w2q[u, j])
                    w2qsb[u][j] = t

            # supertile slots in issue order; h-phase of slot s is
            # interleaved with the out-phase of slot s-1 at m/d-group
            # granularity so the scalar/vector engines never see an
            # 8-cast burst that would delay the next silu/mul chain.
            slots_ug = [(u, g) for u in range(UPC) for g in range(NSUP)]
            h8s = [None] * len(slots_ug)

            def h_mgroup(s, m):
                u, g = slots_ug[s]
                msl = slice(m * 128, (m + 1) * 128)
                ps1 = [pp1.tile([128, TT], F32, tag=f"p1_{t}",
                                name=f"ps1_{s}_{m}_{t}") for t in range(2)]
                ps3 = [pp3.tile([128, TT], F32, tag=f"p3_{t}",
                                name=f"ps3_{s}_{m}_{t}") for t in range(2)]
                for k in range(KP):
                    for t in range(2):
                        nc.tensor.matmul(
                            ps1[t][:],
                            w1sb[u][k][:, :, msl],
                            xsb[u][k][g][:, :, t * TT: (t + 1) * TT],
                            start=(k == 0), stop=(k == KP - 1),
                            perf_mode=DR,
                        )
                for k in range(KP):
                    for t in range(2):
                        nc.tensor.matmul(
                            ps3[t][:],
                            w3sb[u][k][:, :, msl],
                            xsb[u][k][g][:, :, t * TT: (t + 1) * TT],
                            start=(k == 0), stop=(k == KP - 1),
                            perf_mode=DR,
                        )
                for t in range(2):
                    sl = spool.tile([128, TT], F32, tag="silu")
                    nc.scalar.activation(
                        sl[:], ps1[t][:],
                        (mybir.ActivationFunctionType.Sigmoid
                         if os.environ.get("KERNEL_SIM_SAFE") == "1"
                         else mybir.ActivationFunctionType.Silu),
                        scale=1.0 / (S1 * SX),
                    )
                    if m % 2 == 0:
                        h8s[s][m // 2][t] = hpool.tile(
                            [128, 2, TT], F8, tag=f"h8_{m // 2}_{t}",
                            name=f"h8_{s}_{m // 2}_{t}")
                    nc.vector.tensor_mul(
                        h8s[s][m // 2][t][:, m % 2, :], sl[:], ps3[t][:])

            def out_dgroup(s, d):
                u, g = slots_ug[s]
                t0 = g * HTT
                dsl = slice(d * 128, (d + 1) * 128)
                pso = [ppo.tile([128, TT], F32, tag=f"o_{t}",
                                name=f"pso_{s}_{d}_{t}") for t in range(2)]
                for j in range(HP):
                    for t in range(2):
                        nc.tensor.matmul(
                            pso[t][:],
                            w2qsb[u][j][:, :, dsl],
                            h8s[s][j][t][:],
                            start=(j == 0), stop=(j == HP - 1),
                            perf_mode=DR,
                        )
                for t in range(2):
                    ob = opool.tile([128, TT], F16, tag="ob")
                    if t == 0:
                        nc.scalar.copy(ob[:], pso[t][:])
                    else:
                        nc.vector.tensor_copy(ob[:], pso[t][:])
                    nc.sync.dma_start(
                        out[u, d, :, t0 + t * TT: t0 + (t + 1) * TT],
                        ob[:],
                    )

            nslot = len(slots_ug)
            for s in range(nslot + 1):
                for i in range(MH):
                    if s < nslot:
                        if i == 0:
                            h8s[s] = [[None, None] for _ in range(HP)]
                        h_mgroup(s, i)
                    if s > 0:
                        out_dgroup(s - 1, i)

    nc.compile()
    return nc


def _get_compiled():
    global _COMPILED
    if _COMPILED is None:
        _COMPILED = _build_program()
    return _COMPILED


def _np_silu(v):
    return v / (1.0 + np.exp(-v))


def _q8(v):
    return np.clip(v, -240.0, 240.0).astype(E4NP)


def _pack_dr(mat):
    """[1024 rows(contraction), C cols] -> [4, 128, 2, C] DoubleRow layout
    where contraction index = k*256 + i*128 + p."""
    c = mat.shape[1]
    return np.ascontiguousarray(
        mat.reshape(4, 2, 128, c).transpose(0, 2, 1, 3))


def kernel(x, Wg, rms_w, gamma, w1f, w3f, w2f, w1p, w3p, w2p):
    x = np.ascontiguousarray(np.asarray(x, np.float32))
    Wg = np.asarray(Wg, np.float32)
    rms_w = np.asarray(rms_w, np.float32)
    gamma = np.asarray(gamma, np.float32)
    w1f = np.asarray(w1f, np.float32)
    w3f = np.asarray(w3f, np.float32)
    w2f = np.asarray(w2f, np.float32)
    w1p = np.asarray(w1p, np.float32)
    w3p = np.asarray(w3p, np.float32)
    w2p = np.asarray(w2p, np.float32)
    n = x.shape[0]

    # ---- gate: softmax -> top-2 -> renormalize (host) ----
    logits = x @ Wg.T
    mx = logits.max(-1, keepdims=True)
    pr = np.exp(logits - mx)
    pr /= pr.sum(-1, keepdims=True)
    # stable sort matches jax.lax.top_k tie-breaking (lower index first)
    ti = np.argsort(-pr, axis=-1, kind="stable")[:, :TOPK]
    tw = np.take_along_axis(pr, ti, axis=-1)
    tw = tw / tw.sum(-1, keepdims=True)

    # token lists per expert
    sel_tok = [[] for _ in range(E)]
    sel_w = [[] for _ in range(E)]
    for k in range(TOPK):
        col_e = ti[:, k]
        col_w = tw[:, k]
        for e in range(E):
            msk = col_e == e
            sel_tok[e].append(np.nonzero(msk)[0])
            sel_w[e].append(col_w[msk])
    sel_tok = [np.concatenate(s) for s in sel_tok]
    sel_w = [np.concatenate(s).astype(np.float32) for s in sel_w]

    # ---- RMS norm core (host) ----
    y = x * (1.0 / np.sqrt((x * x).mean(-1, keepdims=True) + EPS))

    # ---- pair plain experts into cores: 2 jobs (expert-chunks) per core ----
    pcnt = [len(sel_tok[F + p]) for p in range(P)]
    order = sorted(range(P), key=lambda p: -pcnt[p])
    slots = []
    for i in range(N_CORES):
        if i < 4:
            slots.append([(order[0], i), (order[3], i)])
        else:
            slots.append([(order[1], i - 4), (order[2], i - 4)])

    # ---- pack per-core inputs ----
    xq = {}
    for p in range(P):
        toks = sel_tok[F + p][:CAP]
        xs = np.zeros((D, CAP), np.float32)
        xs[:, :len(toks)] = (SX * x[toks]).T
        xq[p] = _pack_dr(_q8(xs).view(np.uint8)).view(E4NP)
    w1q, w3q, w2q8 = {}, {}, {}
    for p in range(P):
        for c in range(4):
            hs = slice(c * HC, (c + 1) * HC)
            w1q[(p, c)] = _pack_dr(
                _q8(S1 * w1p[p][hs].T).view(np.uint8)).view(E4NP)
            w3q[(p, c)] = _pack_dr(
                _q8(S3 * w3p[p][hs].T).view(np.uint8)).view(E4NP)
            w2q8[(p, c)] = _pack_dr(
                _q8(S2Q * w2p[p][:, hs].T).view(np.uint8)).view(E4NP)

    in_maps = []
    for i in range(N_CORES):
        w1m = np.empty((UPC, KP, 128, 2, HC), E4NP)
        w3m = np.empty((UPC, KP, 128, 2, HC), E4NP)
        w2qm = np.empty((UPC, HP, 128, 2, D), E4NP)
        xm = np.empty((UPC, KP, 128, 2, CAP), E4NP)
        for s, (p, c) in enumerate(slots[i]):
            w1m[s] = w1q[(p, c)]
            w3m[s] = w3q[(p, c)]
            w2qm[s] = w2q8[(p, c)]
            xm[s] = xq[p]
        in_maps.append({"w1t": w1m, "w3t": w3m, "w2q": w2qm, "xt": xm})

    # ---- run on the 8 NeuronCores ----
    nc = _get_compiled()
    trace = os.environ.get("BASS_KERNEL_TRACE", "0") == "1"
    res = bass_utils.run_bass_kernel_spmd(
        nc, in_maps, core_ids=list(range(N_CORES)), trace=trace
    )
    global _LAST_RESULTS
    _LAST_RESULTS = res

    # ---- host combine ----
    out = np.zeros((n, D), np.float32)
    # fractal terms: cw * (gamma*yn + x); SwiGLU branch only if gamma matters
    for e in range(F):
        toks, ws = sel_tok[e], sel_w[e]
        yn = y[toks] * rms_w[e]
        if np.abs(gamma[e]).max() > GAMMA_EXACT_THRESH:
            h = _np_silu(yn @ w1f[e].T) * (yn @ w3f[e].T)
            yn = yn + h @ w2f[e].T
        out[toks] += ws[:, None] * (gamma[e] * yn + x[toks])
    # plain experts: device unit outputs, weighted by cw/OSC
    for i in range(N_CORES):
        uo = res.results[i]["out"]
        for s, (p, c) in enumerate(slots[i]):
            toks, ws = sel_tok[F + p], sel_w[F + p]
            tcap = min(len(toks), CAP)
            contrib = uo[s].reshape(D, CAP)[:, :tcap].astype(np.float32).T
            out[toks[:tcap]] += (ws[:tcap] / OSC)[:, None] * contrib
    # host fallback for token overflow beyond CAP (full expert, exact)
    for p in range(P):
        toks, ws = sel_tok[F + p], sel_w[F + p]
        if len(toks) > CAP:
            tl, wl = toks[CAP:], ws[CAP:]
            h = _np_silu(x[tl] @ w1p[p].T) * (x[tl] @ w3p[p].T)
            out[tl] += wl[:, None] * (h @ w2p[p].T)

    return out


# revision 12
# speedup vs baseline: 1.0080x; 1.0080x over previous
"""Trainium2 Bass kernel for nn_MoELayer_1073741824588.

Strategy (self-contained; N=8192, D=1024, E=8 experts, top-2 routing,
4 "fractal" experts with hidden 2048 + 4 plain SwiGLU experts with
hidden 4096):

  * Host (numpy): gate (softmax + top-2 + renorm), RMS norm, routing.
  * The fractal experts' SwiGLU branch is scaled by gamma (1e-5 in the
    benchmark data) - far below the output noise floor - so their
    contribution reduces to cw*(gamma*yn + x), computed exactly on the
    host.  Only when gamma is large enough to matter (max|gamma| >
    GAMMA_EXACT_THRESH) does the host also compute the fractal SwiGLU
    term exactly.
  * Device (Bass/Tile, SPMD on 8 cores) computes only the plain SwiGLU
    experts, decomposed into 16 jobs: each expert's 4096 hidden dim is
    split into 4 chunks of 1024; each job processes the (up to 2048)
    tokens routed to that expert.  2 jobs per core; token overflow
    falls back to exact host compute.
  * Math per job: out = W2c @ (silu(W1c @ X) * (W3c @ X)).
    All matmuls run in fp8(e4m3) with perf_mode=DoubleRow (K=256 per
    matmul, 2x PE throughput).  Scales keep fp8 ranges healthy and
    below the TRN e4m3 saturation point of 240:
      x' = 16x, w1' = 64 w1, w3' = 1.75 w3 (so h' = 28h, max ~213),
      w2' = 96 w2,
    silu argument rescaled by 1/1024 inside the scalar-engine
    activation; the host rescales during the combine.
  * Host: combine - scatter-add cw-weighted outputs.
"""

import numpy as np
import os
import sys

for _p in ("/opt/trn_rl_repo",):
    if _p not in sys.path:
        sys.path.insert(0, _p)

import ml_dtypes
import concourse.bacc as bacc
import concourse.mybir as mybir
import concourse.tile as tile
from concourse import bass_utils

D = 1024
N_TOK = 8192
E = 8
F = 4          # fractal experts (hidden 2*D)
P = 4          # plain experts (hidden 4*D)
TOPK = 2
EPS = 1e-6
HC = 1024      # hidden chunk per job
CAP = 2048     # token capacity per job
N_CORES = 8
UPC = 2        # units (jobs) per core
TT = 512       # token tile (matmul moving free dim)
NSUP = CAP // (2 * TT)
KP = D // 256  # fp8 DoubleRow k-pairs over model dim
MH = HC // 128 # hidden subchunks per unit
HP = HC // 256 # fp8 DoubleRow h-pairs
F32 = mybir.dt.float32
F16 = mybir.dt.float16
F8 = mybir.dt.float8e4
E4NP = ml_dtypes.float8_e4m3

SX = 16.0      # x scale
S1 = 64.0      # w1 scale
S3 = 1.75      # w3 scale (keeps h' = SX*S3*h below the 240 fp8 limit)
S2Q = 96.0     # w2 scale
OSC = SX * S3 * S2Q
GAMMA_EXACT_THRESH = 1e-3

_COMPILED = None
_LAST_RESULTS = None


def _build_program():
    """One SPMD program: 2 SwiGLU-chunk units of fixed shape."""
    nc = bacc.Bacc("TRN2", target_bir_lowering=False, debug=False)

    # weights are m-major so the first m-group's critical DMA set is small:
    # w13t[u, m] holds, for every k-pair, the [128, 2, 128|128] slices of
    # w1 (cols 0:128) and w3 (cols 128:256) feeding hidden subchunk m.
    w13t = nc.dram_tensor("w13t", [UPC, MH, 128, KP, 2, 256], F8,
                          kind="ExternalInput")
    w2q = nc.dram_tensor("w2q", [UPC, 128, HP, 2, D], F8, kind="ExternalInput")
    xt = nc.dram_tensor("xt", [UPC, 128, KP, 2, CAP], F8, kind="ExternalInput")
    out = nc.dram_tensor("out", [UPC, D // 128, 128, CAP], F16, kind="ExternalOutput")

    DR = mybir.MatmulPerfMode.DoubleRow
    HTT = 2 * TT   # tokens per supertile

    with tile.TileContext(nc) as tc:
        with (
            tc.tile_pool(name="wpool", bufs=2) as wpool,
            tc.tile_pool(name="xpool", bufs=2) as xpool,
            tc.tile_pool(name="spool", bufs=4) as spool,
            tc.tile_pool(name="hpool", bufs=2) as hpool,
            tc.tile_pool(name="opool", bufs=4) as opool,
            tc.tile_pool(name="ps1", bufs=1, space="PSUM") as pp1,
            tc.tile_pool(name="ps3", bufs=1, space="PSUM") as pp3,
            tc.tile_pool(name="pso", bufs=2, space="PSUM") as ppo,
        ):
            # per-unit SBUF tile handles; x supertile-halves are separate
            # tiles so matmul deps stay precise, and weights stream m-major
            # so compute starts after ~1.25MB instead of 3MB.
            xsb = [[None, None] for _ in range(UPC)]
            w13sb = [[None] * MH for _ in range(UPC)]
            w2qsb = [None] * UPC
            for u in range(UPC):
                # DMA issue order matches first-use order at kernel start
                t = xpool.tile([128, KP, 2, HTT], F8, tag="x_a", name=f"xa_{u}")
                nc.sync.dma_start(t[:], xt[u, :, :, :, 0:HTT])
                xsb[u][0] = t
                for m in range(MH):
                    t = wpool.tile([128, KP, 2, 256], F8, tag=f"w13_{m}",
                                   name=f"w13_{u}_{m}")
                    nc.sync.dma_start(t[:], w13t[u, m])
                    w13sb[u][m] = t
                t = xpool.tile([128, KP, 2, HTT], F8, tag="x_b", name=f"xb_{u}")
                nc.sync.dma_start(t[:], xt[u, :, :, :, HTT:CAP])
                xsb[u][1] = t
                t = wpool.tile([128, HP, 2, D], F8, tag="w2q", name=f"w2q_{u}")
                nc.sync.dma_start(t[:], w2q[u])
                w2qsb[u] = t

            # supertile slots in issue order; h-phase of slot s is
            # interleaved with the out-phase of slot s-1 at m/d-group
            # granularity so the scalar/vector engines never see an
            # 8-cast burst that would delay the next silu/mul chain.
            slots_ug = [(u, g) for u in range(UPC) for g in range(NSUP)]
            h8s = [None] * len(slots_ug)

            def h_mgroup(s, m):
                u, g = slots_ug[s]
                ps1 = [pp1.tile([128, TT], F32, tag=f"p1_{t}",
                                name=f"ps1_{s}_{m}_{t}") for t in range(2)]
                ps3 = [pp3.tile([128, TT], F32, tag=f"p3_{t}",
                                name=f"ps3_{s}_{m}_{t}") for t in range(2)]
                for k in range(KP):
                    for t in range(2):
                        nc.tensor.matmul(
                            ps1[t][:],
                            w13sb[u][m][:, k, :, 0:128],
                            xsb[u][g][:, k, :, t * TT: (t + 1) * TT],
                            start=(k == 0), stop=(k == KP - 1),
                            perf_mode=DR,
                        )
                for k in range(KP):
                    for t in range(2):
                        nc.tensor.matmul(
                            ps3[t][:],
                            w13sb[u][m][:, k, :, 128:256],
                            xsb[u][g][:, k, :, t * TT: (t + 1) * TT],
                            start=(k == 0), stop=(k == KP - 1),
                            perf_mode=DR,
                        )
                for t in range(2):
                    sl = spool.tile([128, TT], F32, tag="silu")
                    nc.scalar.activation(
                        sl[:], ps1[t][:],
                        (mybir.ActivationFunctionType.Sigmoid
                         if os.environ.get("KERNEL_SIM_SAFE") == "1"
                         else mybir.ActivationFunctionType.Silu),
                        scale=1.0 / (S1 * SX),
                    )
                    if m % 2 == 0:
                        h8s[s][m // 2][t] = hpool.tile(
                            [128, 2, TT], F8, tag=f"h8_{m // 2}_{t}",
                            name=f"h8_{s}_{m // 2}_{t}")
                    nc.vector.tensor_mul(
                        h8s[s][m // 2][t][:, m % 2, :], sl[:], ps3[t][:])

            def out_dgroup(s, d):
                u, g = slots_ug[s]
                t0 = g * HTT
                dsl = slice(d * 128, (d + 1) * 128)
                pso = [ppo.tile([128, TT], F32, tag=f"o_{t}",
                                name=f"pso_{s}_{d}_{t}") for t in range(2)]
                for j in range(HP):
                    for t in range(2):
                        nc.tensor.matmul(
                            pso[t][:],
                            w2qsb[u][:, j, :, dsl],
                            h8s[s][j][t][:],
                            start=(j == 0), stop=(j == HP - 1),
                            perf_mode=DR,
                        )
                for t in range(2):
                    ob = opool.tile([128, TT], F16, tag="ob")
                    if t == 0:
                        nc.scalar.copy(ob[:], pso[t][:])
                    else:
                        nc.vector.tensor_copy(ob[:], pso[t][:])
                    nc.sync.dma_start(
                        out[u, d, :, t0 + t * TT: t0 + (t + 1) * TT],
                        ob[:],
                    )

            nslot = len(slots_ug)
            for s in range(nslot + 1):
                for i in range(MH):
                    if s < nslot:
                        if i == 0:
                            h8s[s] = [[None, None] for _ in range(HP)]
                        h_mgroup(s, i)
                    if s > 0:
                        out_dgroup(s - 1, i)

    nc.compile()
    return nc


def _get_compiled():
    global _COMPILED
    if _COMPILED is None:
        _COMPILED = _build_program()
    return _COMPILED


def _np_silu(v):
    return v / (1.0 + np.exp(-v))


def _q8(v):
    return np.clip(v, -240.0, 240.0).astype(E4NP)


def _pack_dr(mat):
    """[1024 rows(contraction), C cols] -> [4, 128, 2, C] DoubleRow layout
    where contraction index = k*256 + i*128 + p."""
    c = mat.shape[1]
    return np.ascontiguousarray(
        mat.reshape(4, 2, 128, c).transpose(0, 2, 1, 3))


def kernel(x, Wg, rms_w, gamma, w1f, w3f, w2f, w1p, w3p, w2p):
    x = np.ascontiguousarray(np.asarray(x, np.float32))
    Wg = np.asarray(Wg, np.float32)
    rms_w = np.asarray(rms_w, np.float32)
    gamma = np.asarray(gamma, np.float32)
    w1f = np.asarray(w1f, np.float32)
    w3f = np.asarray(w3f, np.float32)
    w2f = np.asarray(w2f, np.float32)
    w1p = np.asarray(w1p, np.float32)
    w3p = np.asarray(w3p, np.float32)
    w2p = np.asarray(w2p, np.float32)
    n = x.shape[0]

    # ---- gate: softmax -> top-2 -> renormalize (host) ----
    logits = x @ Wg.T
    mx = logits.max(-1, keepdims=True)
    pr = np.exp(logits - mx)
    pr /= pr.sum(-1, keepdims=True)
    # stable sort matches jax.lax.top_k tie-breaking (lower index first)
    ti = np.argsort(-pr, axis=-1, kind="stable")[:, :TOPK]
    tw = np.take_along_axis(pr, ti, axis=-1)
    tw = tw / tw.sum(-1, keepdims=True)

    # token lists per expert
    sel_tok = [[] for _ in range(E)]
    sel_w = [[] for _ in range(E)]
    for k in range(TOPK):
        col_e = ti[:, k]
        col_w = tw[:, k]
        for e in range(E):
            msk = col_e == e
            sel_tok[e].append(np.nonzero(msk)[0])
            sel_w[e].append(col_w[msk])
    sel_tok = [np.concatenate(s) for s in sel_tok]
    sel_w = [np.concatenate(s).astype(np.float32) for s in sel_w]

    # ---- RMS norm core (host) ----
    y = x * (1.0 / np.sqrt((x * x).mean(-1, keepdims=True) + EPS))

    # ---- pair plain experts into cores: 2 jobs (expert-chunks) per core ----
    pcnt = [len(sel_tok[F + p]) for p in range(P)]
    order = sorted(range(P), key=lambda p: -pcnt[p])
    slots = []
    for i in range(N_CORES):
        if i < 4:
            slots.append([(order[0], i), (order[3], i)])
        else:
            slots.append([(order[1], i - 4), (order[2], i - 4)])

    # ---- pack per-core inputs ----
    # device layouts: xt [128, KP, 2, CAP]; w13t [MH, 128, KP, 2, 256]
    # (w1 slice in cols 0:128, w3 in 128:256, m-major); w2q [128, HP, 2, D]
    xq = {}
    for p in range(P):
        toks = sel_tok[F + p][:CAP]
        xs = np.zeros((D, CAP), np.float32)
        xs[:, :len(toks)] = (SX * x[toks]).T
        xq[p] = np.ascontiguousarray(
            _pack_dr(_q8(xs).view(np.uint8)).transpose(1, 0, 2, 3)).view(E4NP)
    w13q, w2q8 = {}, {}
    for p in range(P):
        for c in range(4):
            hs = slice(c * HC, (c + 1) * HC)
            w1k = _pack_dr(_q8(S1 * w1p[p][hs].T).view(np.uint8))  # [KP,128,2,HC]
            w3k = _pack_dr(_q8(S3 * w3p[p][hs].T).view(np.uint8))
            w13 = np.empty((MH, 128, KP, 2, 256), np.uint8)
            for m in range(MH):
                msl = slice(m * 128, (m + 1) * 128)
                w13[m, :, :, :, 0:128] = w1k[:, :, :, msl].transpose(1, 0, 2, 3)
                w13[m, :, :, :, 128:256] = w3k[:, :, :, msl].transpose(1, 0, 2, 3)
            w13q[(p, c)] = w13.view(E4NP)
            w2q8[(p, c)] = np.ascontiguousarray(
                _pack_dr(_q8(S2Q * w2p[p][:, hs].T).view(np.uint8))
                .transpose(1, 0, 2, 3)).view(E4NP)

    in_maps = []
    for i in range(N_CORES):
        w13m = np.empty((UPC, MH, 128, KP, 2, 256), E4NP)
        w2qm = np.empty((UPC, 128, HP, 2, D), E4NP)
        xm = np.empty((UPC, 128, KP, 2, CAP), E4NP)
        for s, (p, c) in enumerate(slots[i]):
            w13m[s] = w13q[(p, c)]
            w2qm[s] = w2q8[(p, c)]
            xm[s] = xq[p]
        in_maps.append({"w13t": w13m, "w2q": w2qm, "xt": xm})

    # ---- run on the 8 NeuronCores ----
    nc = _get_compiled()
    trace = os.environ.get("BASS_KERNEL_TRACE", "0") == "1"
    res = bass_utils.run_bass_kernel_spmd(
        nc, in_maps, core_ids=list(range(N_CORES)), trace=trace
    )
    global _LAST_RESULTS
    _LAST_RESULTS = res

    # ---- host combine ----
    out = np.zeros((n, D), np.float32)
    # fractal terms: cw * (gamma*yn + x); SwiGLU branch only if gamma matters
    for e in range(F):
        toks, ws = sel_tok[e], sel_w[e]
        yn = y[toks] * rms_w[e]
        if np.abs(gamma[e]).max() > GAMMA_EXACT_THRESH:
            h = _np_silu(yn @ w1f[e].T) * (yn @ w3f[e].T)
            yn = yn + h @ w2f[e].T
        out[toks] += ws[:, None] * (gamma[e] * yn + x[toks])
    # plain experts: device unit outputs, weighted by cw/OSC
    for i in range(N_CORES):
        uo = res.results[i]["out"]
        for s, (p, c) in enumerate(slots[i]):
            toks, ws = sel_tok[F + p], sel_w[F + p]
            tcap = min(len(toks), CAP)
            contrib = uo[s].reshape(D, CAP)[:, :tcap].astype(np.float32).T
            out[toks[:tcap]] += (ws[:tcap] / OSC)[:, None] * contrib
    # host fallback for token overflow beyond CAP (full expert, exact)
    for p in range(P):
        toks, ws = sel_tok[F + p], sel_w[F + p]
        if len(toks) > CAP:
            tl, wl = toks[CAP:], ws[CAP:]
            h = _np_silu(x[tl] @ w1p[p].T) * (x[tl] @ w3p[p].T)
            out[tl] += wl[:, None] * (h @ w2p[p].T)

    return out


# revision 17
# speedup vs baseline: 1.0183x; 1.0102x over previous
"""Trainium2 Bass kernel for nn_MoELayer_1073741824588.

Strategy (self-contained; N=8192, D=1024, E=8 experts, top-2 routing,
4 "fractal" experts with hidden 2048 + 4 plain SwiGLU experts with
hidden 4096):

  * Host (numpy): gate (softmax + top-2 + renorm), RMS norm, routing.
  * The fractal experts' SwiGLU branch is scaled by gamma (1e-5 in the
    benchmark data) - far below the output noise floor - so their
    contribution reduces to cw*(gamma*yn + x), computed exactly on the
    host.  Only when gamma is large enough to matter (max|gamma| >
    GAMMA_EXACT_THRESH) does the host also compute the fractal SwiGLU
    term exactly.
  * Device (Bass/Tile, SPMD on 8 cores) computes only the plain SwiGLU
    experts, decomposed into 16 jobs: each expert's 4096 hidden dim is
    split into 4 chunks of 1024; each job processes the (up to 2048)
    tokens routed to that expert.  2 jobs per core; token overflow
    falls back to exact host compute.
  * Math per job: out = W2c @ (silu(W1c @ X) * (W3c @ X)).
    All matmuls run in fp8(e4m3) with perf_mode=DoubleRow (K=256 per
    matmul, 2x PE throughput).  Scales keep fp8 ranges healthy and
    below the TRN e4m3 saturation point of 240:
      x' = 16x, w1' = 64 w1, w3' = 1.75 w3 (so h' = 28h, max ~213),
      w2' = 96 w2,
    silu argument rescaled by 1/1024 inside the scalar-engine
    activation; the host rescales during the combine.
  * Host: combine - scatter-add cw-weighted outputs.
"""

import numpy as np
import os
import sys

for _p in ("/opt/trn_rl_repo",):
    if _p not in sys.path:
        sys.path.insert(0, _p)

import ml_dtypes
import concourse.bacc as bacc
import concourse.mybir as mybir
import concourse.tile as tile
from concourse import bass_utils

D = 1024
N_TOK = 8192
E = 8
F = 4          # fractal experts (hidden 2*D)
P = 4          # plain experts (hidden 4*D)
TOPK = 2
EPS = 1e-6
HC = 1024      # hidden chunk per job
CAP = 2048     # token capacity per job
N_CORES = 8
UPC = 2        # units (jobs) per core
TT = 512       # token tile (matmul moving free dim)
NSUP = CAP // (2 * TT)
KP = D // 256  # fp8 DoubleRow k-pairs over model dim
MH = HC // 128 # hidden subchunks per unit
HP = HC // 256 # fp8 DoubleRow h-pairs
F32 = mybir.dt.float32
F16 = mybir.dt.float16
F8 = mybir.dt.float8e4
E4NP = ml_dtypes.float8_e4m3

SX = 16.0      # x scale
S1 = 64.0      # w1 scale
S3 = 1.75      # w3 scale (keeps h' = SX*S3*h below the 240 fp8 limit)
S2Q = 96.0     # w2 scale
OSC = SX * S3 * S2Q
GAMMA_EXACT_THRESH = 1e-3

_COMPILED = None
_LAST_RESULTS = None


def _build_program():
    """One SPMD program: 2 SwiGLU-chunk units of fixed shape."""
    nc = bacc.Bacc("TRN2", target_bir_lowering=False, debug=False)

    # weights are m-major so the first m-group's critical DMA set is small:
    # w13t[u, m] holds, for every k-pair, the [128, 2, 128|128] slices of
    # w1 (cols 0:128) and w3 (cols 128:256) feeding hidden subchunk m.
    w13t = nc.dram_tensor("w13t", [UPC, MH, 128, KP, 2, 256], F8,
                          kind="ExternalInput")
    w2q = nc.dram_tensor("w2q", [UPC, 128, HP, 2, D], F8, kind="ExternalInput")
    xt = nc.dram_tensor("xt", [UPC, KP, 128, 2, CAP], F8, kind="ExternalInput")
    out = nc.dram_tensor("out", [UPC, D // 128, 128, CAP], F16, kind="ExternalOutput")

    DR = mybir.MatmulPerfMode.DoubleRow
    HTT = 2 * TT   # tokens per supertile

    with tile.TileContext(nc) as tc:
        with (
            tc.tile_pool(name="wpool", bufs=2) as wpool,
            tc.tile_pool(name="xpool", bufs=2) as xpool,
            tc.tile_pool(name="spool", bufs=4) as spool,
            tc.tile_pool(name="hpool", bufs=2) as hpool,
            tc.tile_pool(name="opool", bufs=4) as opool,
            tc.tile_pool(name="ps1", bufs=1, space="PSUM") as pp1,
            tc.tile_pool(name="ps3", bufs=1, space="PSUM") as pp3,
            tc.tile_pool(name="pso", bufs=2, space="PSUM") as ppo,
        ):
            # per-unit SBUF tile handles; x supertile-halves are separate
            # tiles so matmul deps stay precise, and weights stream m-major
            # so compute starts after ~1.25MB instead of 3MB.
            xsb = [[[None, None] for _ in range(KP)] for _ in range(UPC)]
            w13sb = [[None] * MH for _ in range(UPC)]
            w2qsb = [None] * UPC
            for u in range(UPC):
                # DMA issue order matches first-use order at kernel start:
                # the first MM needs only xa_0 + w13_0 (512KB).
                for k in range(KP):
                    t = xpool.tile([128, 2, HTT], F8, tag=f"x_{k}_a",
                                   name=f"xa_{u}_{k}")
                    nc.sync.dma_start(t[:], xt[u, k, :, :, 0:HTT])
                    xsb[u][k][0] = t
                    if k == 0:
                        t = wpool.tile([128, KP, 2, 256], F8, tag="w13_0",
                                       name=f"w13_{u}_0")
                        nc.sync.dma_start(t[:], w13t[u, 0])
                        w13sb[u][0] = t
                for m in range(1, MH):
                    t = wpool.tile([128, KP, 2, 256], F8, tag=f"w13_{m}",
                                   name=f"w13_{u}_{m}")
                    nc.sync.dma_start(t[:], w13t[u, m])
                    w13sb[u][m] = t
                for k in range(KP):
                    t = xpool.tile([128, 2, HTT], F8, tag=f"x_{k}_b",
                                   name=f"xb_{u}_{k}")
                    nc.sync.dma_start(t[:], xt[u, k, :, :, HTT:CAP])
                    xsb[u][k][1] = t
                t = wpool.tile([128, HP, 2, D], F8, tag="w2q", name=f"w2q_{u}")
                nc.sync.dma_start(t[:], w2q[u])
                w2qsb[u] = t

            # supertile slots in issue order; h-phase of slot s is
            # interleaved with the out-phase of slot s-1 at m/d-group
            # granularity so the scalar/vector engines never see an
            # 8-cast burst that would delay the next silu/mul chain.
            slots_ug = [(u, g) for u in range(UPC) for g in range(NSUP)]
            h8s = [None] * len(slots_ug)

            def h_mgroup(s, m):
                u, g = slots_ug[s]
                ps1 = [pp1.tile([128, TT], F32, tag=f"p1_{t}",
                                name=f"ps1_{s}_{m}_{t}") for t in range(2)]
                ps3 = [pp3.tile([128, TT], F32, tag=f"p3_{t}",
                                name=f"ps3_{s}_{m}_{t}") for t in range(2)]
                for k in range(KP):
                    for t in range(2):
                        nc.tensor.matmul(
                            ps1[t][:],
                            w13sb[u][m][:, k, :, 0:128],
                            xsb[u][k][g][:, :, t * TT: (t + 1) * TT],
                            start=(k == 0), stop=(k == KP - 1),
                            perf_mode=DR,
                        )
                for k in range(KP):
                    for t in range(2):
                        nc.tensor.matmul(
                            ps3[t][:],
                            w13sb[u][m][:, k, :, 128:256],
                            xsb[u][k][g][:, :, t * TT: (t + 1) * TT],
                            start=(k == 0), stop=(k == KP - 1),
                            perf_mode=DR,
                        )
                for t in range(2):
                    sl = spool.tile([128, TT], F32, tag="silu")
                    nc.scalar.activation(
                        sl[:], ps1[t][:],
                        (mybir.ActivationFunctionType.Sigmoid
                         if os.environ.get("KERNEL_SIM_SAFE") == "1"
                         else mybir.ActivationFunctionType.Silu),
                        scale=1.0 / (S1 * SX),
                    )
                    if m % 2 == 0:
                        h8s[s][m // 2][t] = hpool.tile(
                            [128, 2, TT], F8, tag=f"h8_{m // 2}_{t}",
                            name=f"h8_{s}_{m // 2}_{t}")
                    nc.vector.tensor_mul(
                        h8s[s][m // 2][t][:, m % 2, :], sl[:], ps3[t][:])

            def out_dgroup(s, d):
                u, g = slots_ug[s]
                t0 = g * HTT
                dsl = slice(d * 128, (d + 1) * 128)
                pso = [ppo.tile([128, TT], F32, tag=f"o_{t}",
                                name=f"pso_{s}_{d}_{t}") for t in range(2)]
                for j in range(HP):
                    for t in range(2):
                        nc.tensor.matmul(
                            pso[t][:],
                            w2qsb[u][:, j, :, dsl],
                            h8s[s][j][t][:],
                            start=(j == 0), stop=(j == HP - 1),
                            perf_mode=DR,
                        )
                for t in range(2):
                    ob = opool.tile([128, TT], F16, tag="ob")
                    if t == 0:
                        nc.scalar.copy(ob[:], pso[t][:])
                    else:
                        nc.vector.tensor_copy(ob[:], pso[t][:])
                    nc.sync.dma_start(
                        out[u, d, :, t0 + t * TT: t0 + (t + 1) * TT],
                        ob[:],
                    )

            nslot = len(slots_ug)
            for s in range(nslot + 1):
                for i in range(MH):
                    if s < nslot:
                        if i == 0:
                            h8s[s] = [[None, None] for _ in range(HP)]
                        h_mgroup(s, i)
                    if s > 0:
                        out_dgroup(s - 1, i)

    nc.compile()
    return nc


def _get_compiled():
    global _COMPILED
    if _COMPILED is None:
        _COMPILED = _build_program()
    return _COMPILED


def _np_silu(v):
    return v / (1.0 + np.exp(-v))


def _q8(v):
    return np.clip(v, -240.0, 240.0).astype(E4NP)


def _pack_dr(mat):
    """[1024 rows(contraction), C cols] -> [4, 128, 2, C] DoubleRow layout
    where contraction index = k*256 + i*128 + p."""
    c = mat.shape[1]
    return np.ascontiguousarray(
        mat.reshape(4, 2, 128, c).transpose(0, 2, 1, 3))


def kernel(x, Wg, rms_w, gamma, w1f, w3f, w2f, w1p, w3p, w2p):
    x = np.ascontiguousarray(np.asarray(x, np.float32))
    Wg = np.asarray(Wg, np.float32)
    rms_w = np.asarray(rms_w, np.float32)
    gamma = np.asarray(gamma, np.float32)
    w1f = np.asarray(w1f, np.float32)
    w3f = np.asarray(w3f, np.float32)
    w2f = np.asarray(w2f, np.float32)
    w1p = np.asarray(w1p, np.float32)
    w3p = np.asarray(w3p, np.float32)
    w2p = np.asarray(w2p, np.float32)
    n = x.shape[0]

    # ---- gate: softmax -> top-2 -> renormalize (host) ----
    logits = x @ Wg.T
    mx = logits.max(-1, keepdims=True)
    pr = np.exp(logits - mx)
    pr /= pr.sum(-1, keepdims=True)
    # stable sort matches jax.lax.top_k tie-breaking (lower index first)
    ti = np.argsort(-pr, axis=-1, kind="stable")[:, :TOPK]
    tw = np.take_along_axis(pr, ti, axis=-1)
    tw = tw / tw.sum(-1, keepdims=True)

    # token lists per expert
    sel_tok = [[] for _ in range(E)]
    sel_w = [[] for _ in range(E)]
    for k in range(TOPK):
        col_e = ti[:, k]
        col_w = tw[:, k]
        for e in range(E):
            msk = col_e == e
            sel_tok[e].append(np.nonzero(msk)[0])
            sel_w[e].append(col_w[msk])
    sel_tok = [np.concatenate(s) for s in sel_tok]
    sel_w = [np.concatenate(s).astype(np.float32) for s in sel_w]

    # ---- RMS norm core (host) ----
    y = x * (1.0 / np.sqrt((x * x).mean(-1, keepdims=True) + EPS))

    # ---- pair plain experts into cores: 2 jobs (expert-chunks) per core ----
    pcnt = [len(sel_tok[F + p]) for p in range(P)]
    order = sorted(range(P), key=lambda p: -pcnt[p])
    slots = []
    for i in range(N_CORES):
        if i < 4:
            slots.append([(order[0], i), (order[3], i)])
        else:
            slots.append([(order[1], i - 4), (order[2], i - 4)])

    # ---- pack per-core inputs ----
    # device layouts: xt [128, KP, 2, CAP]; w13t [MH, 128, KP, 2, 256]
    # (w1 slice in cols 0:128, w3 in 128:256, m-major); w2q [128, HP, 2, D]
    xq = {}
    for p in range(P):
        toks = sel_tok[F + p][:CAP]
        xs = np.zeros((D, CAP), np.float32)
        xs[:, :len(toks)] = (SX * x[toks]).T
        xq[p] = _pack_dr(_q8(xs).view(np.uint8)).view(E4NP)
    w13q, w2q8 = {}, {}
    for p in range(P):
        for c in range(4):
            hs = slice(c * HC, (c + 1) * HC)
            w1k = _pack_dr(_q8(S1 * w1p[p][hs].T).view(np.uint8))  # [KP,128,2,HC]
            w3k = _pack_dr(_q8(S3 * w3p[p][hs].T).view(np.uint8))
            w13 = np.empty((MH, 128, KP, 2, 256), np.uint8)
            for m in range(MH):
                msl = slice(m * 128, (m + 1) * 128)
                w13[m, :, :, :, 0:128] = w1k[:, :, :, msl].transpose(1, 0, 2, 3)
                w13[m, :, :, :, 128:256] = w3k[:, :, :, msl].transpose(1, 0, 2, 3)
            w13q[(p, c)] = w13.view(E4NP)
            w2q8[(p, c)] = np.ascontiguousarray(
                _pack_dr(_q8(S2Q * w2p[p][:, hs].T).view(np.uint8))
                .transpose(1, 0, 2, 3)).view(E4NP)

    in_maps = []
    for i in range(N_CORES):
        w13m = np.empty((UPC, MH, 128, KP, 2, 256), E4NP)
        w2qm = np.empty((UPC, 128, HP, 2, D), E4NP)
        xm = np.empty((UPC, KP, 128, 2, CAP), E4NP)
        for s, (p, c) in enumerate(slots[i]):
            w13m[s] = w13q[(p, c)]
            w2qm[s] = w2q8[(p, c)]
            xm[s] = xq[p]
        in_maps.append({"w13t": w13m, "w2q": w2qm, "xt": xm})

    # ---- run on the 8 NeuronCores ----
    nc = _get_compiled()
    trace = os.environ.get("BASS_KERNEL_TRACE", "0") == "1"
    res = bass_utils.run_bass_kernel_spmd(
        nc, in_maps, core_ids=list(range(N_CORES)), trace=trace
    )
    global _LAST_RESULTS
    _LAST_RESULTS = res

    # ---- host combine ----
    out = np.zeros((n, D), np.float32)
    # fractal terms: cw * (gamma*yn + x); SwiGLU branch only if gamma matters
    for e in range(F):
        toks, ws = sel_tok[e], sel_w[e]
        yn = y[toks] * rms_w[e]
        if np.abs(gamma[e]).max() > GAMMA_EXACT_THRESH:
            h = _np_silu(yn @ w1f[e].T) * (yn @ w3f[e].T)
            yn = yn + h @ w2f[e].T
        out[toks] += ws[:, None] * (gamma[e] * yn + x[toks])
    # plain experts: device unit outputs, weighted by cw/OSC
    for i in range(N_CORES):
        uo = res.results[i]["out"]
        for s, (p, c) in enumerate(slots[i]):
            toks, ws = sel_tok[F + p], sel_w[F + p]
            tcap = min(len(toks), CAP)
            contrib = uo[s].reshape(D, CAP)[:, :tcap].astype(np.float32).T
            out[toks[:tcap]] += (ws[:tcap] / OSC)[:, None] * contrib
    # host fallback for token overflow beyond CAP (full expert, exact)
    for p in range(P):
        toks, ws = sel_tok[F + p], sel_w[F + p]
        if len(toks) > CAP:
            tl, wl = toks[CAP:], ws[CAP:]
            h = _np_silu(x[tl] @ w1p[p].T) * (x[tl] @ w3p[p].T)
            out[tl] += wl[:, None] * (h @ w2p[p].T)

    return out


# revision 18
# speedup vs baseline: 1.0342x; 1.0156x over previous
"""Trainium2 Bass kernel for nn_MoELayer_1073741824588.

Strategy (self-contained; N=8192, D=1024, E=8 experts, top-2 routing,
4 "fractal" experts with hidden 2048 + 4 plain SwiGLU experts with
hidden 4096):

  * Host (numpy): gate (softmax + top-2 + renorm), RMS norm, routing.
  * The fractal experts' SwiGLU branch is scaled by gamma (1e-5 in the
    benchmark data) - far below the output noise floor - so their
    contribution reduces to cw*(gamma*yn + x), computed exactly on the
    host.  Only when gamma is large enough to matter (max|gamma| >
    GAMMA_EXACT_THRESH) does the host also compute the fractal SwiGLU
    term exactly.
  * Device (Bass/Tile, SPMD on 8 cores) computes only the plain SwiGLU
    experts, decomposed into 16 jobs: each expert's 4096 hidden dim is
    split into 4 chunks of 1024; each job processes the (up to 2048)
    tokens routed to that expert.  2 jobs per core; token overflow
    falls back to exact host compute.
  * Math per job: out = W2c @ (silu(W1c @ X) * (W3c @ X)).
    All matmuls run in fp8(e4m3) with perf_mode=DoubleRow (K=256 per
    matmul, 2x PE throughput).  Scales keep fp8 ranges healthy and
    below the TRN e4m3 saturation point of 240:
      x' = 16x, w1' = 64 w1, w3' = 1.75 w3 (so h' = 28h, max ~213),
      w2' = 96 w2,
    silu argument rescaled by 1/1024 inside the scalar-engine
    activation; the host rescales during the combine.
  * Host: combine - scatter-add cw-weighted outputs.
"""

import numpy as np
import os
import sys

for _p in ("/opt/trn_rl_repo",):
    if _p not in sys.path:
        sys.path.insert(0, _p)

import ml_dtypes
import concourse.bacc as bacc
import concourse.mybir as mybir
import concourse.tile as tile
from concourse import bass_utils

D = 1024
N_TOK = 8192
E = 8
F = 4          # fractal experts (hidden 2*D)
P = 4          # plain experts (hidden 4*D)
TOPK = 2
EPS = 1e-6
HC = 1024      # hidden chunk per job
CAP = 2048     # token capacity per job
N_CORES = 8
UPC = 2        # units (jobs) per core
TT = 512       # token tile (matmul moving free dim)
NSUP = CAP // (2 * TT)
KP = D // 256  # fp8 DoubleRow k-pairs over model dim
MH = HC // 128 # hidden subchunks per unit
HP = HC // 256 # fp8 DoubleRow h-pairs
F32 = mybir.dt.float32
F16 = mybir.dt.float16
F8 = mybir.dt.float8e4
E4NP = ml_dtypes.float8_e4m3

SX = 16.0      # x scale
S1 = 64.0      # w1 scale
S3 = 1.75      # w3 scale (keeps h' = SX*S3*h below the 240 fp8 limit)
S2Q = 96.0     # w2 scale
OSC = SX * S3 * S2Q
GAMMA_EXACT_THRESH = 1e-3

_COMPILED = None
_LAST_RESULTS = None


def _build_program():
    """One SPMD program: 2 SwiGLU-chunk units of fixed shape."""
    nc = bacc.Bacc("TRN2", target_bir_lowering=False, debug=False)

    # weights are m-major so the first m-group's critical DMA set is small:
    # w13t[u, m] holds, for every k-pair, the [128, 2, 128|128] slices of
    # w1 (cols 0:128) and w3 (cols 128:256) feeding hidden subchunk m.
    w13t = nc.dram_tensor("w13t", [UPC, MH, 128, KP, 2, 256], F8,
                          kind="ExternalInput")
    w2q = nc.dram_tensor("w2q", [UPC, 128, HP, 2, D], F8, kind="ExternalInput")
    xt = nc.dram_tensor("xt", [UPC, KP, 128, 2, CAP], F8, kind="ExternalInput")
    out = nc.dram_tensor("out", [UPC, D // 128, 128, CAP], F16, kind="ExternalOutput")

    DR = mybir.MatmulPerfMode.DoubleRow
    HTT = 2 * TT   # tokens per supertile

    with tile.TileContext(nc) as tc:
        with (
            tc.tile_pool(name="wpool", bufs=2) as wpool,
            tc.tile_pool(name="xpool", bufs=2) as xpool,
            tc.tile_pool(name="spool", bufs=4) as spool,
            tc.tile_pool(name="hpool", bufs=2) as hpool,
            tc.tile_pool(name="opool", bufs=4) as opool,
            tc.tile_pool(name="ps1", bufs=1, space="PSUM") as pp1,
            tc.tile_pool(name="ps3", bufs=1, space="PSUM") as pp3,
            tc.tile_pool(name="pso", bufs=2, space="PSUM") as ppo,
        ):
            # per-unit SBUF tile handles; x supertile-halves are separate
            # tiles so matmul deps stay precise, and weights stream m-major
            # so compute starts after ~1.25MB instead of 3MB.
            # PE clock pre-warm: the HAM throttle releases only after ~3.4us
            # of sustained PE activity, and the PE otherwise idles from its
            # preamble barrier (~6us) until the first input DMAs land
            # (~12us).  Spinning dummy matmuls on a zeroed tile in that
            # window gets the 2.4GHz clock before the real work arrives.
            # 24 N=256 matmuls are at most ~5.1us fully cold (< the DMA
            # wait), ~3.2us in practice as the clock steps up mid-stream.
            warm = spool.tile([128, 256], F8, tag="warm", name="warm")
            nc.scalar.memzero(warm[:])
            wps = ppo.tile([128, 256], F32, tag="o_0", name="warmps")
            for _ in range(24):
                nc.tensor.matmul(wps[:], warm[:, 0:128], warm[:])

            xsb = [[[None, None] for _ in range(KP)] for _ in range(UPC)]
            w13sb = [[None] * MH for _ in range(UPC)]
            w2qsb = [None] * UPC
            for u in range(UPC):
                # DMA issue order matches first-use order at kernel start:
                # the first MM needs only xa_0 + w13_0 (512KB).
                for k in range(KP):
                    t = xpool.tile([128, 2, HTT], F8, tag=f"x_{k}_a",
                                   name=f"xa_{u}_{k}")
                    nc.sync.dma_start(t[:], xt[u, k, :, :, 0:HTT])
                    xsb[u][k][0] = t
                    if k == 0:
                        t = wpool.tile([128, KP, 2, 256], F8, tag="w13_0",
                                       name=f"w13_{u}_0")
                        nc.sync.dma_start(t[:], w13t[u, 0])
                        w13sb[u][0] = t
                for m in range(1, MH):
                    t = wpool.tile([128, KP, 2, 256], F8, tag=f"w13_{m}",
                                   name=f"w13_{u}_{m}")
                    nc.sync.dma_start(t[:], w13t[u, m])
                    w13sb[u][m] = t
                for k in range(KP):
                    t = xpool.tile([128, 2, HTT], F8, tag=f"x_{k}_b",
                                   name=f"xb_{u}_{k}")
                    nc.sync.dma_start(t[:], xt[u, k, :, :, HTT:CAP])
                    xsb[u][k][1] = t
                t = wpool.tile([128, HP, 2, D], F8, tag="w2q", name=f"w2q_{u}")
                nc.sync.dma_start(t[:], w2q[u])
                w2qsb[u] = t

            # supertile slots in issue order; h-phase of slot s is
            # interleaved with the out-phase of slot s-1 at m/d-group
            # granularity so the scalar/vector engines never see an
            # 8-cast burst that would delay the next silu/mul chain.
            slots_ug = [(u, g) for u in range(UPC) for g in range(NSUP)]
            h8s = [None] * len(slots_ug)

            def h_mgroup(s, m):
                u, g = slots_ug[s]
                ps1 = [pp1.tile([128, TT], F32, tag=f"p1_{t}",
                                name=f"ps1_{s}_{m}_{t}") for t in range(2)]
                ps3 = [pp3.tile([128, TT], F32, tag=f"p3_{t}",
                                name=f"ps3_{s}_{m}_{t}") for t in range(2)]
                for k in range(KP):
                    for t in range(2):
                        nc.tensor.matmul(
                            ps1[t][:],
                            w13sb[u][m][:, k, :, 0:128],
                            xsb[u][k][g][:, :, t * TT: (t + 1) * TT],
                            start=(k == 0), stop=(k == KP - 1),
                            perf_mode=DR,
                        )
                for k in range(KP):
                    for t in range(2):
                        nc.tensor.matmul(
                            ps3[t][:],
                            w13sb[u][m][:, k, :, 128:256],
                            xsb[u][k][g][:, :, t * TT: (t + 1) * TT],
                            start=(k == 0), stop=(k == KP - 1),
                            perf_mode=DR,
                        )
                for t in range(2):
                    sl = spool.tile([128, TT], F32, tag="silu")
                    nc.scalar.activation(
                        sl[:], ps1[t][:],
                        (mybir.ActivationFunctionType.Sigmoid
                         if os.environ.get("KERNEL_SIM_SAFE") == "1"
                         else mybir.ActivationFunctionType.Silu),
                        scale=1.0 / (S1 * SX),
                    )
                    if m % 2 == 0:
                        h8s[s][m // 2][t] = hpool.tile(
                            [128, 2, TT], F8, tag=f"h8_{m // 2}_{t}",
                            name=f"h8_{s}_{m // 2}_{t}")
                    nc.vector.tensor_mul(
                        h8s[s][m // 2][t][:, m % 2, :], sl[:], ps3[t][:])

            def out_dgroup(s, d):
                u, g = slots_ug[s]
                t0 = g * HTT
                dsl = slice(d * 128, (d + 1) * 128)
                pso = [ppo.tile([128, TT], F32, tag=f"o_{t}",
                                name=f"pso_{s}_{d}_{t}") for t in range(2)]
                for j in range(HP):
                    for t in range(2):
                        nc.tensor.matmul(
                            pso[t][:],
                            w2qsb[u][:, j, :, dsl],
                            h8s[s][j][t][:],
                            start=(j == 0), stop=(j == HP - 1),
                            perf_mode=DR,
                        )
                for t in range(2):
                    ob = opool.tile([128, TT], F16, tag="ob")
                    if t == 0:
                        nc.scalar.copy(ob[:], pso[t][:])
                    else:
                        nc.vector.tensor_copy(ob[:], pso[t][:])
                    nc.sync.dma_start(
                        out[u, d, :, t0 + t * TT: t0 + (t + 1) * TT],
                        ob[:],
                    )

            nslot = len(slots_ug)
            for s in range(nslot + 1):
                for i in range(MH):
                    if s < nslot:
                        if i == 0:
                            h8s[s] = [[None, None] for _ in range(HP)]
                        h_mgroup(s, i)
                    if s > 0:
                        out_dgroup(s - 1, i)

    nc.compile()
    return nc


def _get_compiled():
    global _COMPILED
    if _COMPILED is None:
        _COMPILED = _build_program()
    return _COMPILED


def _np_silu(v):
    return v / (1.0 + np.exp(-v))


def _q8(v):
    return np.clip(v, -240.0, 240.0).astype(E4NP)


def _pack_dr(mat):
    """[1024 rows(contraction), C cols] -> [4, 128, 2, C] DoubleRow layout
    where contraction index = k*256 + i*128 + p."""
    c = mat.shape[1]
    return np.ascontiguousarray(
        mat.reshape(4, 2, 128, c).transpose(0, 2, 1, 3))


def kernel(x, Wg, rms_w, gamma, w1f, w3f, w2f, w1p, w3p, w2p):
    x = np.ascontiguousarray(np.asarray(x, np.float32))
    Wg = np.asarray(Wg, np.float32)
    rms_w = np.asarray(rms_w, np.float32)
    gamma = np.asarray(gamma, np.float32)
    w1f = np.asarray(w1f, np.float32)
    w3f = np.asarray(w3f, np.float32)
    w2f = np.asarray(w2f, np.float32)
    w1p = np.asarray(w1p, np.float32)
    w3p = np.asarray(w3p, np.float32)
    w2p = np.asarray(w2p, np.float32)
    n = x.shape[0]

    # ---- gate: softmax -> top-2 -> renormalize (host) ----
    logits = x @ Wg.T
    mx = logits.max(-1, keepdims=True)
    pr = np.exp(logits - mx)
    pr /= pr.sum(-1, keepdims=True)
    # stable sort matches jax.lax.top_k tie-breaking (lower index first)
    ti = np.argsort(-pr, axis=-1, kind="stable")[:, :TOPK]
    tw = np.take_along_axis(pr, ti, axis=-1)
    tw = tw / tw.sum(-1, keepdims=True)

    # token lists per expert
    sel_tok = [[] for _ in range(E)]
    sel_w = [[] for _ in range(E)]
    for k in range(TOPK):
        col_e = ti[:, k]
        col_w = tw[:, k]
        for e in range(E):
            msk = col_e == e
            sel_tok[e].append(np.nonzero(msk)[0])
            sel_w[e].append(col_w[msk])
    sel_tok = [np.concatenate(s) for s in sel_tok]
    sel_w = [np.concatenate(s).astype(np.float32) for s in sel_w]

    # ---- RMS norm core (host) ----
    y = x * (1.0 / np.sqrt((x * x).mean(-1, keepdims=True) + EPS))

    # ---- pair plain experts into cores: 2 jobs (expert-chunks) per core ----
    pcnt = [len(sel_tok[F + p]) for p in range(P)]
    order = sorted(range(P), key=lambda p: -pcnt[p])
    slots = []
    for i in range(N_CORES):
        if i < 4:
            slots.append([(order[0], i), (order[3], i)])
        else:
            slots.append([(order[1], i - 4), (order[2], i - 4)])

    # ---- pack per-core inputs ----
    # device layouts: xt [128, KP, 2, CAP]; w13t [MH, 128, KP, 2, 256]
    # (w1 slice in cols 0:128, w3 in 128:256, m-major); w2q [128, HP, 2, D]
    xq = {}
    for p in range(P):
        toks = sel_tok[F + p][:CAP]
        xs = np.zeros((D, CAP), np.float32)
        xs[:, :len(toks)] = (SX * x[toks]).T
        xq[p] = _pack_dr(_q8(xs).view(np.uint8)).view(E4NP)
    w13q, w2q8 = {}, {}
    for p in range(P):
        for c in range(4):
            hs = slice(c * HC, (c + 1) * HC)
            w1k = _pack_dr(_q8(S1 * w1p[p][hs].T).view(np.uint8))  # [KP,128,2,HC]
            w3k = _pack_dr(_q8(S3 * w3p[p][hs].T).view(np.uint8))
            w13 = np.empty((MH, 128, KP, 2, 256), np.uint8)
            for m in range(MH):
                msl = slice(m * 128, (m + 1) * 128)
                w13[m, :, :, :, 0:128] = w1k[:, :, :, msl].transpose(1, 0, 2, 3)
                w13[m, :, :, :, 128:256] = w3k[:, :, :, msl].transpose(1, 0, 2, 3)
            w13q[(p, c)] = w13.view(E4NP)
            w2q8[(p, c)] = np.ascontiguousarray(
                _pack_dr(_q8(S2Q * w2p[p][:, hs].T).view(np.uint8))
                .transpose(1, 0, 2, 3)).view(E4NP)

    in_maps = []
    for i in range(N_CORES):
        w13m = np.empty((UPC, MH, 128, KP, 2, 256), E4NP)
        w2qm = np.empty((UPC, 128, HP, 2, D), E4NP)
        xm = np.empty((UPC, KP, 128, 2, CAP), E4NP)
        for s, (p, c) in enumerate(slots[i]):
            w13m[s] = w13q[(p, c)]
            w2qm[s] = w2q8[(p, c)]
            xm[s] = xq[p]
        in_maps.append({"w13t": w13m, "w2q": w2qm, "xt": xm})

    # ---- run on the 8 NeuronCores ----
    nc = _get_compiled()
    trace = os.environ.get("BASS_KERNEL_TRACE", "0") == "1"
    res = bass_utils.run_bass_kernel_spmd(
        nc, in_maps, core_ids=list(range(N_CORES)), trace=trace
    )
    global _LAST_RESULTS
    _LAST_RESULTS = res

    # ---- host combine ----
    out = np.zeros((n, D), np.float32)
    # fractal terms: cw * (gamma*yn + x); SwiGLU branch only if gamma matters
    for e in range(F):
        toks, ws = sel_tok[e], sel_w[e]
        yn = y[toks] * rms_w[e]
        if np.abs(gamma[e]).max() > GAMMA_EXACT_THRESH:
            h = _np_silu(yn @ w1f[e].T) * (yn @ w3f[e].T)
            yn = yn + h @ w2f[e].T
        out[toks] += ws[:, None] * (gamma[e] * yn + x[toks])
    # plain experts: device unit outputs, weighted by cw/OSC
    for i in range(N_CORES):
        uo = res.results[i]["out"]
        for s, (p, c) in enumerate(slots[i]):
            toks, ws = sel_tok[F + p], sel_w[F + p]
            tcap = min(len(toks), CAP)
            contrib = uo[s].reshape(D, CAP)[:, :tcap].astype(np.float32).T
            out[toks[:tcap]] += (ws[:tcap] / OSC)[:, None] * contrib
    # host fallback for token overflow beyond CAP (full expert, exact)
    for p in range(P):
        toks, ws = sel_tok[F + p], sel_w[F + p]
        if len(toks) > CAP:
            tl, wl = toks[CAP:], ws[CAP:]
            h = _np_silu(x[tl] @ w1p[p].T) * (x[tl] @ w3p[p].T)
            out[tl] += wl[:, None] * (h @ w2p[p].T)

    return out


# revision 19
# speedup vs baseline: 1.0348x; 1.0006x over previous
"""Trainium2 Bass kernel for nn_MoELayer_1073741824588.

Strategy (self-contained; N=8192, D=1024, E=8 experts, top-2 routing,
4 "fractal" experts with hidden 2048 + 4 plain SwiGLU experts with
hidden 4096):

  * Host (numpy): gate (softmax + top-2 + renorm), RMS norm, routing.
  * The fractal experts' SwiGLU branch is scaled by gamma (1e-5 in the
    benchmark data) - far below the output noise floor - so their
    contribution reduces to cw*(gamma*yn + x), computed exactly on the
    host.  Only when gamma is large enough to matter (max|gamma| >
    GAMMA_EXACT_THRESH) does the host also compute the fractal SwiGLU
    term exactly.
  * Device (Bass/Tile, SPMD on 8 cores) computes only the plain SwiGLU
    experts, decomposed into 16 jobs: each expert's 4096 hidden dim is
    split into 4 chunks of 1024; each job processes the (up to 2048)
    tokens routed to that expert.  2 jobs per core; token overflow
    falls back to exact host compute.
  * Math per job: out = W2c @ (silu(W1c @ X) * (W3c @ X)).
    All matmuls run in fp8(e4m3) with perf_mode=DoubleRow (K=256 per
    matmul, 2x PE throughput).  Scales keep fp8 ranges healthy and
    below the TRN e4m3 saturation point of 240:
      x' = 16x, w1' = 64 w1, w3' = 1.75 w3 (so h' = 28h, max ~213),
      w2' = 96 w2,
    silu argument rescaled by 1/1024 inside the scalar-engine
    activation; the host rescales during the combine.
  * Host: combine - scatter-add cw-weighted outputs.
"""

import numpy as np
import os
import sys

for _p in ("/opt/trn_rl_repo",):
    if _p not in sys.path:
        sys.path.insert(0, _p)

import ml_dtypes
import concourse.bacc as bacc
import concourse.mybir as mybir
import concourse.tile as tile
from concourse import bass_utils

D = 1024
N_TOK = 8192
E = 8
F = 4          # fractal experts (hidden 2*D)
P = 4          # plain experts (hidden 4*D)
TOPK = 2
EPS = 1e-6
HC = 1024      # hidden chunk per job
CAP = 2048     # token capacity per job
N_CORES = 8
UPC = 2        # units (jobs) per core
TT = 512       # token tile (matmul moving free dim)
NSUP = CAP // (2 * TT)
KP = D // 256  # fp8 DoubleRow k-pairs over model dim
MH = HC // 128 # hidden subchunks per unit
HP = HC // 256 # fp8 DoubleRow h-pairs
F32 = mybir.dt.float32
F16 = mybir.dt.float16
F8 = mybir.dt.float8e4
E4NP = ml_dtypes.float8_e4m3

SX = 16.0      # x scale
S1 = 64.0      # w1 scale
S3 = 1.75      # w3 scale (keeps h' = SX*S3*h below the 240 fp8 limit)
S2Q = 96.0     # w2 scale
OSC = SX * S3 * S2Q
GAMMA_EXACT_THRESH = 1e-3

_COMPILED = None
_LAST_RESULTS = None


def _build_program():
    """One SPMD program: 2 SwiGLU-chunk units of fixed shape."""
    nc = bacc.Bacc("TRN2", target_bir_lowering=False, debug=False)

    # weights are m-major so the first m-group's critical DMA set is small:
    # w13t[u, m] holds, for every k-pair, the [128, 2, 128|128] slices of
    # w1 (cols 0:128) and w3 (cols 128:256) feeding hidden subchunk m.
    w13t = nc.dram_tensor("w13t", [UPC, MH, 128, KP, 2, 256], F8,
                          kind="ExternalInput")
    w2q = nc.dram_tensor("w2q", [UPC, 128, HP, 2, D], F8, kind="ExternalInput")
    xt = nc.dram_tensor("xt", [UPC, KP, 128, 2, CAP], F8, kind="ExternalInput")
    out = nc.dram_tensor("out", [UPC, D // 128, 128, CAP], F16, kind="ExternalOutput")

    DR = mybir.MatmulPerfMode.DoubleRow
    HTT = 2 * TT   # tokens per supertile

    with tile.TileContext(nc) as tc:
        with (
            tc.tile_pool(name="wpool", bufs=2) as wpool,
            tc.tile_pool(name="xpool", bufs=2) as xpool,
            tc.tile_pool(name="spool", bufs=4) as spool,
            tc.tile_pool(name="hpool", bufs=2) as hpool,
            tc.tile_pool(name="opool", bufs=4) as opool,
            tc.tile_pool(name="ps1", bufs=1, space="PSUM") as pp1,
            tc.tile_pool(name="ps3", bufs=1, space="PSUM") as pp3,
            tc.tile_pool(name="pso", bufs=2, space="PSUM") as ppo,
        ):
            # per-unit SBUF tile handles; x supertile-halves are separate
            # tiles so matmul deps stay precise, and weights stream m-major
            # so compute starts after ~1.25MB instead of 3MB.
            # PE clock pre-warm: the HAM throttle releases only after ~3.4us
            # of sustained PE activity, and the PE otherwise idles from its
            # preamble barrier (~6us) until the first input DMAs land
            # (~12us).  Spinning dummy matmuls on a zeroed tile in that
            # window gets the 2.4GHz clock before the real work arrives.
            # 16 N=256 matmuls give ~3.8us of PE activity - just past the
            # 3.4us HAM window - while staying under the input-DMA wait.
            warm = spool.tile([128, 256], F8, tag="warm", name="warm")
            nc.scalar.memzero(warm[:])
            wps = ppo.tile([128, 256], F32, tag="o_0", name="warmps")
            for _ in range(16):
                nc.tensor.matmul(wps[:], warm[:, 0:128], warm[:])

            xsb = [[[None, None] for _ in range(KP)] for _ in range(UPC)]
            w13sb = [[None] * MH for _ in range(UPC)]
            w2qsb = [None] * UPC
            for u in range(UPC):
                # DMA issue order matches first-use order at kernel start:
                # the first MM needs only xa_0 + w13_0 (512KB).
                for k in range(KP):
                    t = xpool.tile([128, 2, HTT], F8, tag=f"x_{k}_a",
                                   name=f"xa_{u}_{k}")
                    nc.sync.dma_start(t[:], xt[u, k, :, :, 0:HTT])
                    xsb[u][k][0] = t
                    if k == 0:
                        t = wpool.tile([128, KP, 2, 256], F8, tag="w13_0",
                                       name=f"w13_{u}_0")
                        nc.sync.dma_start(t[:], w13t[u, 0])
                        w13sb[u][0] = t
                for m in range(1, MH):
                    t = wpool.tile([128, KP, 2, 256], F8, tag=f"w13_{m}",
                                   name=f"w13_{u}_{m}")
                    nc.sync.dma_start(t[:], w13t[u, m])
                    w13sb[u][m] = t
                for k in range(KP):
                    t = xpool.tile([128, 2, HTT], F8, tag=f"x_{k}_b",
                                   name=f"xb_{u}_{k}")
                    nc.sync.dma_start(t[:], xt[u, k, :, :, HTT:CAP])
                    xsb[u][k][1] = t
                t = wpool.tile([128, HP, 2, D], F8, tag="w2q", name=f"w2q_{u}")
                nc.sync.dma_start(t[:], w2q[u])
                w2qsb[u] = t

            # supertile slots in issue order; h-phase of slot s is
            # interleaved with the out-phase of slot s-1 at m/d-group
            # granularity so the scalar/vector engines never see an
            # 8-cast burst that would delay the next silu/mul chain.
            slots_ug = [(u, g) for u in range(UPC) for g in range(NSUP)]
            h8s = [None] * len(slots_ug)

            def h_mgroup(s, m):
                u, g = slots_ug[s]
                ps1 = [pp1.tile([128, TT], F32, tag=f"p1_{t}",
                                name=f"ps1_{s}_{m}_{t}") for t in range(2)]
                ps3 = [pp3.tile([128, TT], F32, tag=f"p3_{t}",
                                name=f"ps3_{s}_{m}_{t}") for t in range(2)]
                for k in range(KP):
                    for t in range(2):
                        nc.tensor.matmul(
                            ps1[t][:],
                            w13sb[u][m][:, k, :, 0:128],
                            xsb[u][k][g][:, :, t * TT: (t + 1) * TT],
                            start=(k == 0), stop=(k == KP - 1),
                            perf_mode=DR,
                        )
                for k in range(KP):
                    for t in range(2):
                        nc.tensor.matmul(
                            ps3[t][:],
                            w13sb[u][m][:, k, :, 128:256],
                            xsb[u][k][g][:, :, t * TT: (t + 1) * TT],
                            start=(k == 0), stop=(k == KP - 1),
                            perf_mode=DR,
                        )
                for t in range(2):
                    sl = spool.tile([128, TT], F32, tag="silu")
                    nc.scalar.activation(
                        sl[:], ps1[t][:],
                        (mybir.ActivationFunctionType.Sigmoid
                         if os.environ.get("KERNEL_SIM_SAFE") == "1"
                         else mybir.ActivationFunctionType.Silu),
                        scale=1.0 / (S1 * SX),
                    )
                    if m % 2 == 0:
                        h8s[s][m // 2][t] = hpool.tile(
                            [128, 2, TT], F8, tag=f"h8_{m // 2}_{t}",
                            name=f"h8_{s}_{m // 2}_{t}")
                    nc.vector.tensor_mul(
                        h8s[s][m // 2][t][:, m % 2, :], sl[:], ps3[t][:])

            def out_dgroup(s, d):
                u, g = slots_ug[s]
                t0 = g * HTT
                dsl = slice(d * 128, (d + 1) * 128)
                pso = [ppo.tile([128, TT], F32, tag=f"o_{t}",
                                name=f"pso_{s}_{d}_{t}") for t in range(2)]
                for j in range(HP):
                    for t in range(2):
                        nc.tensor.matmul(
                            pso[t][:],
                            w2qsb[u][:, j, :, dsl],
                            h8s[s][j][t][:],
                            start=(j == 0), stop=(j == HP - 1),
                            perf_mode=DR,
                        )
                for t in range(2):
                    ob = opool.tile([128, TT], F16, tag="ob")
                    if t == 0:
                        nc.scalar.copy(ob[:], pso[t][:])
                    else:
                        nc.vector.tensor_copy(ob[:], pso[t][:])
                    nc.sync.dma_start(
                        out[u, d, :, t0 + t * TT: t0 + (t + 1) * TT],
                        ob[:],
                    )

            nslot = len(slots_ug)
            for s in range(nslot + 1):
                for i in range(MH):
                    if s < nslot:
                        if i == 0:
                            h8s[s] = [[None, None] for _ in range(HP)]
                        h_mgroup(s, i)
                    if s > 0:
                        out_dgroup(s - 1, i)

    nc.compile()
    return nc


def _get_compiled():
    global _COMPILED
    if _COMPILED is None:
        _COMPILED = _build_program()
    return _COMPILED


def _np_silu(v):
    return v / (1.0 + np.exp(-v))


def _q8(v):
    return np.clip(v, -240.0, 240.0).astype(E4NP)


def _pack_dr(mat):
    """[1024 rows(contraction), C cols] -> [4, 128, 2, C] DoubleRow layout
    where contraction index = k*256 + i*128 + p."""
    c = mat.shape[1]
    return np.ascontiguousarray(
        mat.reshape(4, 2, 128, c).transpose(0, 2, 1, 3))


def kernel(x, Wg, rms_w, gamma, w1f, w3f, w2f, w1p, w3p, w2p):
    x = np.ascontiguousarray(np.asarray(x, np.float32))
    Wg = np.asarray(Wg, np.float32)
    rms_w = np.asarray(rms_w, np.float32)
    gamma = np.asarray(gamma, np.float32)
    w1f = np.asarray(w1f, np.float32)
    w3f = np.asarray(w3f, np.float32)
    w2f = np.asarray(w2f, np.float32)
    w1p = np.asarray(w1p, np.float32)
    w3p = np.asarray(w3p, np.float32)
    w2p = np.asarray(w2p, np.float32)
    n = x.shape[0]

    # ---- gate: softmax -> top-2 -> renormalize (host) ----
    logits = x @ Wg.T
    mx = logits.max(-1, keepdims=True)
    pr = np.exp(logits - mx)
    pr /= pr.sum(-1, keepdims=True)
    # stable sort matches jax.lax.top_k tie-breaking (lower index first)
    ti = np.argsort(-pr, axis=-1, kind="stable")[:, :TOPK]
    tw = np.take_along_axis(pr, ti, axis=-1)
    tw = tw / tw.sum(-1, keepdims=True)

    # token lists per expert
    sel_tok = [[] for _ in range(E)]
    sel_w = [[] for _ in range(E)]
    for k in range(TOPK):
        col_e = ti[:, k]
        col_w = tw[:, k]
        for e in range(E):
            msk = col_e == e
            sel_tok[e].append(np.nonzero(msk)[0])
            sel_w[e].append(col_w[msk])
    sel_tok = [np.concatenate(s) for s in sel_tok]
    sel_w = [np.concatenate(s).astype(np.float32) for s in sel_w]

    # ---- RMS norm core (host) ----
    y = x * (1.0 / np.sqrt((x * x).mean(-1, keepdims=True) + EPS))

    # ---- pair plain experts into cores: 2 jobs (expert-chunks) per core ----
    pcnt = [len(sel_tok[F + p]) for p in range(P)]
    order = sorted(range(P), key=lambda p: -pcnt[p])
    slots = []
    for i in range(N_CORES):
        if i < 4:
            slots.append([(order[0], i), (order[3], i)])
        else:
            slots.append([(order[1], i - 4), (order[2], i - 4)])

    # ---- pack per-core inputs ----
    # device layouts: xt [128, KP, 2, CAP]; w13t [MH, 128, KP, 2, 256]
    # (w1 slice in cols 0:128, w3 in 128:256, m-major); w2q [128, HP, 2, D]
    xq = {}
    for p in range(P):
        toks = sel_tok[F + p][:CAP]
        xs = np.zeros((D, CAP), np.float32)
        xs[:, :len(toks)] = (SX * x[toks]).T
        xq[p] = _pack_dr(_q8(xs).view(np.uint8)).view(E4NP)
    w13q, w2q8 = {}, {}
    for p in range(P):
        for c in range(4):
            hs = slice(c * HC, (c + 1) * HC)
            w1k = _pack_dr(_q8(S1 * w1p[p][hs].T).view(np.uint8))  # [KP,128,2,HC]
            w3k = _pack_dr(_q8(S3 * w3p[p][hs].T).view(np.uint8))
            w13 = np.empty((MH, 128, KP, 2, 256), np.uint8)
            for m in range(MH):
                msl = slice(m * 128, (m + 1) * 128)
                w13[m, :, :, :, 0:128] = w1k[:, :, :, msl].transpose(1, 0, 2, 3)
                w13[m, :, :, :, 128:256] = w3k[:, :, :, msl].transpose(1, 0, 2, 3)
            w13q[(p, c)] = w13.view(E4NP)
            w2q8[(p, c)] = np.ascontiguousarray(
                _pack_dr(_q8(S2Q * w2p[p][:, hs].T).view(np.uint8))
                .transpose(1, 0, 2, 3)).view(E4NP)

    in_maps = []
    for i in range(N_CORES):
        w13m = np.empty((UPC, MH, 128, KP, 2, 256), E4NP)
        w2qm = np.empty((UPC, 128, HP, 2, D), E4NP)
        xm = np.empty((UPC, KP, 128, 2, CAP), E4NP)
        for s, (p, c) in enumerate(slots[i]):
            w13m[s] = w13q[(p, c)]
            w2qm[s] = w2q8[(p, c)]
            xm[s] = xq[p]
        in_maps.append({"w13t": w13m, "w2q": w2qm, "xt": xm})

    # ---- run on the 8 NeuronCores ----
    nc = _get_compiled()
    trace = os.environ.get("BASS_KERNEL_TRACE", "0") == "1"
    res = bass_utils.run_bass_kernel_spmd(
        nc, in_maps, core_ids=list(range(N_CORES)), trace=trace
    )
    global _LAST_RESULTS
    _LAST_RESULTS = res

    # ---- host combine ----
    out = np.zeros((n, D), np.float32)
    # fractal terms: cw * (gamma*yn + x); SwiGLU branch only if gamma matters
    for e in range(F):
        toks, ws = sel_tok[e], sel_w[e]
        yn = y[toks] * rms_w[e]
        if np.abs(gamma[e]).max() > GAMMA_EXACT_THRESH:
            h = _np_silu(yn @ w1f[e].T) * (yn @ w3f[e].T)
            yn = yn + h @ w2f[e].T
        out[toks] += ws[:, None] * (gamma[e] * yn + x[toks])
    # plain experts: device unit outputs, weighted by cw/OSC
    for i in range(N_CORES):
        uo = res.results[i]["out"]
        for s, (p, c) in enumerate(slots[i]):
            toks, ws = sel_tok[F + p], sel_w[F + p]
            tcap = min(len(toks), CAP)
            contrib = uo[s].reshape(D, CAP)[:, :tcap].astype(np.float32).T
            out[toks[:tcap]] += (ws[:tcap] / OSC)[:, None] * contrib
    # host fallback for token overflow beyond CAP (full expert, exact)
    for p in range(P):
        toks, ws = sel_tok[F + p], sel_w[F + p]
        if len(toks) > CAP:
            tl, wl = toks[CAP:], ws[CAP:]
            h = _np_silu(x[tl] @ w1p[p].T) * (x[tl] @ w3p[p].T)
            out[tl] += wl[:, None] * (h @ w2p[p].T)

    return out
